# revision 1
# baseline (speedup 1.0000x reference)
"""BitAttention TRN2 kernel: 8-core SPMD (DP over batch x TP over kv-heads).

Self-contained: hardcodes shapes B=2, S=2048, D=2048, H=16, KH=4.
Core r: batch b = r//4, kv-head kh = r%4, output token-quarter q# = r%4.

Math (forward-equivalent to the reference):
  - linear_bit = rms_norm -> per-row int8 act quant -> ternary weight quant -> matmul.
    Activations quantize to integers in [-127,127] (exact in bf16); ternary weights
    in {-1,0,1} (exact in bf16) -> projections run as exact-integer bf16 matmuls,
    dequant scales applied at PSUM eviction.
  - The reference einsum sums the query-head group axis, so Q's 16 heads collapse
    to 4 effective heads: group-sum the ternary w_q rows (ints in [-4,4], exact).
  - Both /sqrt(HD) scalings fold into one exact *(1/128) on q.
  - Attention (scores, softmax, P@V) runs in f32.
  - RoPE even/odd pairs are made contiguous by permuting w_q/w_k output dims
    (scores are invariant to a shared permutation of q/k feature dims).
"""
import numpy as np
from contextlib import ExitStack

import concourse.bass as bass
import concourse.bacc as bacc
import concourse.mybir as mybir
import concourse.tile as tile
from concourse.bass_utils import run_bass_kernel_spmd
from concourse.masks import make_identity, make_causal_mask

B, S, D = 2, 2048, 2048
H, KH = 16, 4
HD = D // H          # 128
KVD = KH * HD        # 512
NB = S // 128        # 16 token blocks
SQ = S // 4          # 512 tokens per output quarter
EPS = 1e-8
MAGIC = float(1.5 * 2 ** 23)
ATANH05 = 0.5493061443340549      # arctanh(0.5)
NEG = -3.4e38
F32 = mybir.dt.float32
BF16 = mybir.dt.bfloat16
AX = mybir.AxisListType
OP = mybir.AluOpType
AF = mybir.ActivationFunctionType

_cache = {}


def build(causal: bool, local_cc: bool = False):
    nc = bacc.Bacc()
    x_d = nc.dram_tensor("x", [S, D], F32, kind="ExternalInput")
    wq_d = nc.dram_tensor("wq", [D, KVD], F32, kind="ExternalInput")   # selected+perm+T
    wk_d = nc.dram_tensor("wk", [D, HD], F32, kind="ExternalInput")    # perm+T
    wv_d = nc.dram_tensor("wv", [D, HD], F32, kind="ExternalInput")    # T
    wo_d = nc.dram_tensor("wo", [KVD, D], F32, kind="ExternalInput")   # w_o.T full
    cos_d = nc.dram_tensor("cos", [S, HD // 2], F32, kind="ExternalInput")
    sin_d = nc.dram_tensor("sin", [S, HD // 2], F32, kind="ExternalInput")
    qsel_d = nc.dram_tensor("qsel", [128, 2], F32, kind="ExternalInput")  # quad one-hot
    y_d = nc.dram_tensor("y", [SQ, D], F32, kind="ExternalOutput")
    st_in = nc.dram_tensor("st_in", [1, 4], F32)
    st_out = nc.dram_tensor("st_out", [1, 4], F32, addr_space="Shared")
    cc_in = nc.dram_tensor("cc_in", [8, SQ, HD], F32)
    cc_out = nc.dram_tensor("cc_out", [8, SQ, HD], F32)

    with tile.TileContext(nc) as tc, ExitStack() as ctx:
        cpool = ctx.enter_context(tc.tile_pool(name="const", bufs=1))
        sm = ctx.enter_context(tc.tile_pool(name="sm", bufs=1))
        wint = ctx.enter_context(tc.tile_pool(name="wint", bufs=1))
        psmm = ctx.enter_context(tc.tile_pool(name="psmm", bufs=3, space="PSUM"))
        pstp = ctx.enter_context(tc.tile_pool(name="pstp", bufs=2, space="PSUM"))

        # ---------- constants ----------
        idf = cpool.tile([128, 128], F32, tag="idf")
        make_identity(nc, idf[:])
        idb = cpool.tile([128, 128], BF16, tag="idb")
        make_identity(nc, idb[:])
        eps_t = cpool.tile([128, 1], F32, tag="eps")
        nc.any.memset(eps_t[:], EPS)
        c127 = cpool.tile([128, 1], F32, tag="c127")
        nc.any.memset(c127[:], 127.0)
        ones_c = cpool.tile([128, 1], F32, tag="onc")
        nc.any.memset(ones_c[:], 1.0)
        ones_r = cpool.tile([1, 128], F32, tag="onr")
        nc.any.memset(ones_r[:], 1.0)
        inv_n = cpool.tile([128, 4], F32, tag="invn")
        for j, numel in enumerate([D * D, KVD * D, KVD * D, D * KVD]):
            nc.any.memset(inv_n[:, j:j + 1], 1.0 / (2.0 * numel))
        cmask = cpool.tile([128, 128], F32, tag="cmask")
        if causal:
            make_causal_mask(nc, cmask[:], mask_val=NEG)
        cos_all = cpool.tile([128, NB, HD // 2], F32, tag="cosall")
        sin_all = cpool.tile([128, NB, HD // 2], F32, tag="sinall")
        nc.sync.dma_start(cos_all[:], cos_d.ap().rearrange("(i p) f -> p i f", p=128))
        nc.sync.dma_start(sin_all[:], sin_d.ap().rearrange("(i p) f -> p i f", p=128))

        # persistent small tiles
        deq_all = sm.tile([128, NB], F32, tag="deq_all")
        partials = sm.tile([128, 52], F32, tag="partials")
        ptot = sm.tile([128, 4], F32, tag="ptot")
        st_sb = sm.tile([1, 4], F32, tag="st_sb")
        st2_sb = sm.tile([1, 4], F32, tag="st2_sb")
        totals = sm.tile([128, 4], F32, tag="totals")
        s4 = sm.tile([128, 4], F32, tag="s4")
        thr4 = sm.tile([128, 4], F32, tag="thr4")
        a4 = sm.tile([128, 4], F32, tag="a4")
        aq128 = sm.tile([128, 1], F32, tag="aq128")

        # int weights (persistent)
        wqkv_i = [wint.tile([128, 3 * HD], BF16, tag=f"wi{j}", name=f"wi{j}") for j in range(NB)]
        wo_i = [wint.tile([128, D], BF16, tag=f"wo{c}", name=f"wo{c}") for c in range(4)]

        # ---------- weights pass 1: |w| partial row-sums ----------
        with tc.tile_pool(name="wstream", bufs=4) as wstream:
            def rsum(dst_col, src_ap, w, tagsfx):
                t = wstream.tile([128, w], F32, tag="wst")
                nc.sync.dma_start(t[:, :w], src_ap)
                nc.vector.tensor_reduce(partials[:, dst_col:dst_col + 1], t[:, :w],
                                        axis=AX.X, op=OP.add, apply_absolute_value=True)

            for j in range(NB):
                rsum(j, wq_d[j * 128:(j + 1) * 128, :], KVD, f"q{j}")
            for j in range(NB):
                rsum(16 + j, wk_d[j * 128:(j + 1) * 128, :], HD, f"k{j}")
            for j in range(NB):
                rsum(32 + j, wv_d[j * 128:(j + 1) * 128, :], HD, f"v{j}")
            # w_o: this core's quarter of output dims = columns via qsel mask later;
            # simpler: sum our quarter rows of woT columns [kh*512:(kh+1)*512] is not
            # expressible core-dependently -> host zeroes other quarters? No: host
            # passes identical woT; quarter selection done via per-core input "qsel"
            # would complicate. Instead: every core sums ALL of woT and we divide by
            # 8 (each element counted once per core).
            for c in range(4):
                rsum(48 + c, wo_d[c * 128:(c + 1) * 128, :], D, f"o{c}")

            # segment reductions -> ptot [128,4]
            nc.vector.tensor_reduce(ptot[:, 0:1], partials[:, 0:16], axis=AX.X, op=OP.add)
            nc.vector.tensor_reduce(ptot[:, 1:2], partials[:, 16:32], axis=AX.X, op=OP.add)
            nc.vector.tensor_reduce(ptot[:, 2:3], partials[:, 32:48], axis=AX.X, op=OP.add)
            nc.vector.tensor_reduce(ptot[:, 3:4], partials[:, 48:52], axis=AX.X, op=OP.add)
            # w_o was summed fully on every core: scale its partial by 1/4 so the
            # 8-core AllReduce total equals 2x full-sum like the others
            nc.vector.tensor_scalar(ptot[:, 3:4], ptot[:, 3:4], 0.25, None, op0=OP.mult)
            pcol = psmm.tile([1, 4], F32, tag="mm")
            nc.tensor.matmul(pcol[:], ones_c[:], ptot[:], start=True, stop=True)
            nc.vector.tensor_copy(st_sb[:], pcol[:])
            nc.sync.dma_start(st_in[:], st_sb[:])
            if local_cc:
                nc.sync.dma_start(st_out.ap(), st_in.ap())
            else:
                nc.gpsimd.collective_compute(
                    "AllReduce", OP.add, replica_groups=[list(range(8))],
                    ins=[st_in.ap().opt()], outs=[st_out.ap().opt()])
            nc.sync.dma_start(st2_sb[:], st_out[:])
            bc = psmm.tile([128, 4], F32, tag="mm")
            nc.tensor.matmul(bc[:], ones_r[:], st2_sb[:], start=True, stop=True)
            nc.vector.tensor_copy(totals[:], bc[:])
            # s, thr, a  (all [128,4], replicated across partitions)
            nc.vector.tensor_tensor(s4[:], totals[:], inv_n[:], op=OP.mult)
            nc.vector.tensor_scalar(thr4[:], s4[:], EPS, ATANH05, op0=OP.add, op1=OP.mult)
            num = sm.tile([128, 4], F32, tag="num")
            den = sm.tile([128, 4], F32, tag="den")
            rat = sm.tile([128, 4], F32, tag="rat")
            nc.vector.tensor_scalar(num[:], s4[:], 1.0, None, op0=OP.add)
            nc.vector.tensor_scalar(den[:], s4[:], -1.0, 1.0, op0=OP.mult, op1=OP.add)
            nc.vector.reciprocal(rat[:], den[:])
            ratn = sm.tile([128, 4], F32, tag="ratn")
            nc.vector.tensor_tensor(ratn[:], den[:], rat[:], op=OP.mult)
            nc.vector.tensor_scalar(ratn[:], ratn[:], -1.0, 2.0, op0=OP.mult, op1=OP.add)
            nc.vector.tensor_tensor(rat[:], rat[:], ratn[:], op=OP.mult)
            nc.vector.tensor_tensor(rat[:], rat[:], num[:], op=OP.mult)
            lnr = sm.tile([128, 4], F32, tag="lnr")
            nc.scalar.activation(lnr[:], rat[:], AF.Ln)
            nc.vector.tensor_scalar(a4[:], lnr[:], 0.5, None, op0=OP.mult)
            nc.vector.tensor_scalar(aq128[:], a4[:, 0:1], 1.0 / 128.0, None, op0=OP.mult)
            hi4 = sm.tile([128, 4], F32, tag="hi4")
            nc.vector.reciprocal(hi4[:], thr4[:])
            hin = sm.tile([128, 4], F32, tag="hin")
            nc.vector.tensor_tensor(hin[:], thr4[:], hi4[:], op=OP.mult)
            nc.vector.tensor_scalar(hin[:], hin[:], -1.0, 2.0, op0=OP.mult, op1=OP.add)
            nc.vector.tensor_tensor(hi4[:], hi4[:], hin[:], op=OP.mult)
            nc.vector.tensor_scalar(hi4[:], hi4[:], 0.5, None, op0=OP.mult)

            # ---------- weights pass 2: ternary quantize ----------
            with tc.tile_pool(name="tern", bufs=2) as ternp:
                def ternary(src_ap, w, thr_col, out_ap):
                    # clip(round_half_even(w * 0.5/thr), -1, 1)
                    t = wstream.tile([128, w], F32, tag="wst")
                    nc.sync.dma_start(t[:, :w], src_ap)
                    u = ternp.tile([128, w], F32, tag="u", name="u")
                    nc.vector.tensor_scalar(u[:, :w], t[:, :w],
                                            hi4[:, thr_col:thr_col + 1], MAGIC,
                                            op0=OP.mult, op1=OP.add)
                    nc.vector.tensor_scalar(u[:, :w], u[:, :w], MAGIC, 1.0,
                                            op0=OP.subtract, op1=OP.min)
                    nc.vector.tensor_scalar(out_ap, u[:, :w], -1.0, None,
                                            op0=OP.max)

                for j in range(NB):
                    tq = ternp.tile([128, KVD], BF16, tag="tq")
                    ternary(wq_d[j * 128:(j + 1) * 128, :], KVD, 0, tq[:, :])
                    # group-sum 4 head blocks -> wqkv[:, 0:HD]
                    e1 = ternp.tile([128, HD], BF16, tag="e1")
                    e2 = ternp.tile([128, HD], BF16, tag="e2")
                    nc.vector.tensor_tensor(e1[:], tq[:, 0:HD], tq[:, HD:2 * HD], op=OP.add)
                    nc.vector.tensor_tensor(e2[:], tq[:, 2 * HD:3 * HD], tq[:, 3 * HD:4 * HD], op=OP.add)
                    nc.vector.tensor_tensor(wqkv_i[j][:, 0:HD], e1[:], e2[:], op=OP.add)
                    ternary(wk_d[j * 128:(j + 1) * 128, :], HD, 1, wqkv_i[j][:, HD:2 * HD])
                    ternary(wv_d[j * 128:(j + 1) * 128, :], HD, 2, wqkv_i[j][:, 2 * HD:3 * HD])
                for c in range(4):
                    ternary(wo_d[c * 128:(c + 1) * 128, :], D, 3, wo_i[c][:, :])

        # ---------- x phase: stats + int8 quantize + transpose ----------
        with tc.tile_pool(name="xqTp", bufs=1) as xqTp:
            xqT = xqTp.tile([128, NB, S], BF16, tag="xqT")
            with tc.tile_pool(name="xph", bufs=1) as xph:
                sq_scr = xph.tile([128, D], BF16, tag="sqscr")
                for i in range(NB):
                    xb = xph.tile([128, D], F32, tag="xb", bufs=2)
                    nc.sync.dma_start(xb[:], x_d[i * 128:(i + 1) * 128, :])
                    mx = xph.tile([128, 1], F32, tag="mx", bufs=2)
                    nc.vector.tensor_reduce(mx[:], xb[:], axis=AX.X, op=OP.max,
                                            apply_absolute_value=True)
                    ssq = xph.tile([128, 1], F32, tag="ssq", bufs=2)
                    nc.scalar.activation(sq_scr[:], xb[:], AF.Square, accum_out=ssq[:])
                    mean_t = xph.tile([128, 1], F32, tag="mean_t", bufs=2)
                    nc.vector.tensor_scalar(mean_t[:], ssq[:], 1.0 / D, EPS,
                                            op0=OP.mult, op1=OP.add)
                    sd = xph.tile([128, 1], F32, tag="sd", bufs=2)
                    nc.scalar.activation(sd[:], mean_t[:], AF.Sqrt)
                    r_ = xph.tile([128, 1], F32, tag="r", bufs=2)
                    nc.vector.reciprocal(r_[:], sd[:])
                    nt0 = xph.tile([128, 1], F32, tag="nt0", bufs=2)
                    nc.vector.tensor_tensor(nt0[:], r_[:], r_[:], op=OP.mult)
                    nc.vector.tensor_tensor(nt0[:], nt0[:], mean_t[:], op=OP.mult)
                    nc.vector.tensor_scalar(nt0[:], nt0[:], -0.5, 1.5, op0=OP.mult, op1=OP.add)
                    nc.vector.tensor_tensor(r_[:], r_[:], nt0[:], op=OP.mult)
                    m_ = xph.tile([128, 1], F32, tag="m", bufs=2)
                    nc.vector.tensor_tensor(m_[:], r_[:], mx[:], op=OP.mult)
                    nc.vector.tensor_scalar(m_[:], m_[:], 1e-4, None, op0=OP.max)
                    scl = xph.tile([128, 1], F32, tag="scl", bufs=2)
                    nc.vector.reciprocal(scl[:], m_[:])
                    nt1 = xph.tile([128, 1], F32, tag="nt1", bufs=2)
                    nc.vector.tensor_tensor(nt1[:], m_[:], scl[:], op=OP.mult)
                    nc.vector.tensor_scalar(nt1[:], nt1[:], -1.0, 2.0, op0=OP.mult, op1=OP.add)
                    nc.vector.tensor_tensor(scl[:], scl[:], nt1[:], op=OP.mult)
                    nc.vector.tensor_scalar(scl[:], scl[:], 127.0, None, op0=OP.mult)
                    nc.vector.reciprocal(deq_all[:, i:i + 1], scl[:])
                    nt2 = xph.tile([128, 1], F32, tag="nt2", bufs=2)
                    nc.vector.tensor_tensor(nt2[:], scl[:], deq_all[:, i:i + 1], op=OP.mult)
                    nc.vector.tensor_scalar(nt2[:], nt2[:], -1.0, 2.0, op0=OP.mult, op1=OP.add)
                    nc.vector.tensor_tensor(deq_all[:, i:i + 1], deq_all[:, i:i + 1], nt2[:], op=OP.mult)
                    smul = xph.tile([128, 1], F32, tag="smul", bufs=2)
                    nc.vector.tensor_tensor(smul[:], r_[:], scl[:], op=OP.mult)
                    # in-place: xb = xb*smul + MAGIC ; qb = xb - MAGIC (bf16)
                    nc.vector.tensor_scalar(xb[:], xb[:], smul[:], MAGIC,
                                            op0=OP.mult, op1=OP.add)
                    qb = xph.tile([128, D], BF16, tag="qb", bufs=2)
                    nc.scalar.activation(qb[:], xb[:], AF.Copy, bias=-MAGIC)
                    for jj in range(4):
                        tp = pstp.tile([128, 512], BF16, tag="tp")
                        for u in range(4):
                            j = 4 * jj + u
                            nc.tensor.transpose(tp[:, u * 128:(u + 1) * 128],
                                                qb[:, j * 128:(j + 1) * 128], idb[:])
                        dst = xqT[:, 4 * jj:4 * jj + 4, i * 128:(i + 1) * 128]
                        if jj % 2 == 0:
                            nc.vector.tensor_copy(dst, tp[:])
                        else:
                            nc.scalar.activation(dst, tp[:], AF.Copy)

            # ---------- QKV projections + dequant + rope + transpose ----------
            with tc.tile_pool(name="qkv", bufs=1) as qkv:
                v_all = qkv.tile([128, S], F32, tag="v_all")
                qT = qkv.tile([128, S], F32, tag="qT")
                kT = qkv.tile([128, S], F32, tag="kT")
                for i in range(NB):
                    pq = psmm.tile([128, 3 * HD], F32, tag="mm")
                    for j in range(NB):
                        nc.tensor.matmul(pq[:], xqT[:, j, i * 128:(i + 1) * 128],
                                         wqkv_i[j][:], start=(j == 0), stop=(j == NB - 1))
                    dq = qkv.tile([128, 1], F32, tag="dq", bufs=2)
                    dk = qkv.tile([128, 1], F32, tag="dk", bufs=2)
                    dv = qkv.tile([128, 1], F32, tag="dv", bufs=2)
                    nc.vector.tensor_tensor(dq[:], deq_all[:, i:i + 1], aq128[:], op=OP.mult)
                    nc.vector.tensor_tensor(dk[:], deq_all[:, i:i + 1], a4[:, 1:2], op=OP.mult)
                    nc.vector.tensor_tensor(dv[:], deq_all[:, i:i + 1], a4[:, 2:3], op=OP.mult)
                    qn = qkv.tile([128, HD], F32, tag="qn", bufs=2)
                    kn = qkv.tile([128, HD], F32, tag="kn", bufs=2)
                    nc.scalar.activation(qn[:], pq[:, 0:HD], AF.Copy, scale=dq[:])
                    nc.scalar.activation(kn[:], pq[:, HD:2 * HD], AF.Copy, scale=dk[:])
                    nc.scalar.activation(v_all[:, i * 128:(i + 1) * 128],
                                         pq[:, 2 * HD:3 * HD], AF.Copy, scale=dv[:])
                    # rope (even/odd halves contiguous by host weight permutation)
                    ci = cos_all[:, i, :]
                    si = sin_all[:, i, :]
                    hh = HD // 2
                    qr = qkv.tile([128, HD], F32, tag="qr", bufs=2)
                    kr = qkv.tile([128, HD], F32, tag="kr", bufs=2)
                    for src, dst in ((qn, qr), (kn, kr)):
                        t1 = qkv.tile([128, hh], F32, tag="rt1", bufs=2)
                        t2 = qkv.tile([128, hh], F32, tag="rt2", bufs=2)
                        nc.vector.tensor_tensor(t1[:], src[:, 0:hh], ci, op=OP.mult)
                        nc.vector.tensor_tensor(t2[:], src[:, hh:HD], si, op=OP.mult)
                        nc.vector.tensor_tensor(dst[:, 0:hh], t1[:], t2[:], op=OP.subtract)
                        t3 = qkv.tile([128, hh], F32, tag="rt3", bufs=2)
                        t4 = qkv.tile([128, hh], F32, tag="rt4", bufs=2)
                        nc.vector.tensor_tensor(t3[:], src[:, 0:hh], si, op=OP.mult)
                        nc.vector.tensor_tensor(t4[:], src[:, hh:HD], ci, op=OP.mult)
                        nc.vector.tensor_tensor(dst[:, hh:HD], t3[:], t4[:], op=OP.add)
                    tpq = pstp.tile([128, 512], F32, tag="tp")
                    nc.tensor.transpose(tpq[:, 0:128], qr[:], idf[:])
                    nc.tensor.transpose(tpq[:, 128:256], kr[:], idf[:])
                    nc.vector.tensor_copy(qT[:, i * 128:(i + 1) * 128], tpq[:, 0:128])
                    nc.scalar.activation(kT[:, i * 128:(i + 1) * 128], tpq[:, 128:256],
                                         AF.Copy)

                # ---------- attention ----------
                with tc.tile_pool(name="attn", bufs=1) as attn:
                    for i in range(NB):
                        nk = (i + 1) if causal else NB
                        nch = (nk * 128 + 511) // 512
                        S_sb = attn.tile([128, S], F32, tag="S", bufs=2)
                        zt = attn.tile([128, 4], F32, tag="zt", bufs=2)
                        for c in range(nch):
                            kw = min(512, nk * 128 - c * 512)
                            ps = psmm.tile([128, 512], F32, tag="mm")
                            nc.tensor.matmul(ps[:, :kw], qT[:, i * 128:(i + 1) * 128],
                                             kT[:, c * 512:c * 512 + kw],
                                             start=True, stop=True)
                            if causal and c == nch - 1:
                                nc.vector.tensor_tensor(ps[:, kw - 128:kw],
                                                        ps[:, kw - 128:kw], cmask[:],
                                                        op=OP.add)
                            if c % 2 == 0:
                                nc.vector.tensor_copy(S_sb[:, c * 512:c * 512 + kw],
                                                      ps[:, :kw])
                            else:
                                nc.scalar.activation(S_sb[:, c * 512:c * 512 + kw],
                                                     ps[:, :kw], AF.Copy)
                        mxs = attn.tile([128, 1], F32, tag="mxs", bufs=2)
                        nc.vector.tensor_reduce(mxs[:], S_sb[:, 0:nk * 128], axis=AX.X,
                                                op=OP.max)
                        ngm = attn.tile([128, 1], F32, tag="ngm", bufs=2)
                        nc.vector.tensor_scalar(ngm[:], mxs[:], -1.0, None, op0=OP.mult)
                        for c in range(nch):
                            kw = min(512, nk * 128 - c * 512)
                            nc.scalar.activation(S_sb[:, c * 512:c * 512 + kw],
                                                 S_sb[:, c * 512:c * 512 + kw],
                                                 AF.Exp, bias=ngm[:],
                                                 accum_out=zt[:, c:c + 1])
                        Zi = attn.tile([128, 1], F32, tag="Zi", bufs=2)
                        nc.vector.tensor_reduce(Zi[:], zt[:, 0:nch], axis=AX.X, op=OP.add)
                        rz = attn.tile([128, 1], F32, tag="rz", bufs=2)
                        nc.vector.reciprocal(rz[:], Zi[:])
                        PT = attn.tile([128, S], F32, tag="PT", bufs=2)
                        for kb4 in range((nk + 3) // 4):
                            nkb = min(4, nk - kb4 * 4)
                            tpP = pstp.tile([128, 512], F32, tag="tp")
                            for u in range(nkb):
                                kb = kb4 * 4 + u
                                nc.tensor.transpose(tpP[:, u * 128:(u + 1) * 128],
                                                    S_sb[:, kb * 128:(kb + 1) * 128],
                                                    idf[:])
                            dst = PT[:, kb4 * 512:kb4 * 512 + nkb * 128]
                            if kb4 % 2 == 0:
                                nc.vector.tensor_copy(dst, tpP[:, 0:nkb * 128])
                            else:
                                nc.scalar.activation(dst, tpP[:, 0:nkb * 128], AF.Copy)
                        po = psmm.tile([128, 512], F32, tag="mm")
                        for kb in range(nk):
                            nc.tensor.matmul(po[:, 0:HD], PT[:, kb * 128:(kb + 1) * 128],
                                             v_all[:, kb * 128:(kb + 1) * 128],
                                             start=(kb == 0), stop=(kb == nk - 1))
                        ob = attn.tile([128, HD], F32, tag="ob", bufs=2)
                        nc.scalar.activation(ob[:], po[:, 0:HD], AF.Copy, scale=rz[:])
                        qi = i // 4
                        ro = (i % 4) * 128
                        nc.sync.dma_start(cc_in[qi, ro:ro + 128, :], ob[:])
                        nc.sync.dma_start(cc_in[qi + 4, ro:ro + 128, :], ob[:])

        # ---------- exchange: padded 8-way AllToAll ----------
        if local_cc:
            nc.sync.dma_start(cc_out.ap(), cc_in.ap())
        else:
            nc.gpsimd.collective_compute(
                "AllToAll", OP.bypass, replica_groups=[list(range(8))],
                ins=[cc_in.ap().opt()], outs=[cc_out.ap().opt()])

        # ---------- output projection ----------
        with tc.tile_pool(name="outp", bufs=1) as outp:
            xoT = outp.tile([128, 4, SQ], BF16, tag="xoT")
            osc = outp.tile([128, KVD], BF16, tag="osc")
            qsel = cpool.tile([128, 2], F32, tag="qsel")
            nc.sync.dma_start(qsel[:], qsel_d[:])
            # Receive slots differ per quad (cores 0-3 read A2A slots 0-3, cores
            # 4-7 read slots 4-7) but the program is identical on every core: read
            # all 8 slots and select the right half with a per-core one-hot input.
            for tb in range(4):
                xo8 = outp.tile([128, 8 * HD], F32, tag="xo8", bufs=2)
                src = cc_out.ap()[:, tb * 128:(tb + 1) * 128, :].rearrange(
                    "j p d -> p j d")
                nc.sync.dma_start(xo8[:], src)
                xoa = outp.tile([128, KVD], F32, tag="xoa", bufs=2)
                nc.vector.tensor_scalar(xoa[:], xo8[:, 0:KVD], qsel[:, 0:1], None,
                                        op0=OP.mult)
                xo = outp.tile([128, KVD], F32, tag="xo", bufs=2)
                nc.vector.tensor_scalar(xo[:], xo8[:, KVD:2 * KVD], qsel[:, 1:2],
                                        None, op0=OP.mult)
                nc.vector.tensor_tensor(xo[:], xo[:], xoa[:], op=OP.add)
                mx2 = outp.tile([128, 1], F32, tag="mx2", bufs=2)
                nc.vector.tensor_reduce(mx2[:], xo[:], axis=AX.X, op=OP.max,
                                        apply_absolute_value=True)
                ssq2 = outp.tile([128, 1], F32, tag="ssq2", bufs=2)
                nc.scalar.activation(osc[:], xo[:], AF.Square, accum_out=ssq2[:])
                mean2 = outp.tile([128, 1], F32, tag="mean2", bufs=2)
                nc.vector.tensor_scalar(mean2[:], ssq2[:], 1.0 / KVD, EPS,
                                        op0=OP.mult, op1=OP.add)
                sd2 = outp.tile([128, 1], F32, tag="sd2", bufs=2)
                nc.scalar.activation(sd2[:], mean2[:], AF.Sqrt)
                r2 = outp.tile([128, 1], F32, tag="r2", bufs=2)
                nc.vector.reciprocal(r2[:], sd2[:])
                nt3 = outp.tile([128, 1], F32, tag="nt3", bufs=2)
                nc.vector.tensor_tensor(nt3[:], r2[:], r2[:], op=OP.mult)
                nc.vector.tensor_tensor(nt3[:], nt3[:], mean2[:], op=OP.mult)
                nc.vector.tensor_scalar(nt3[:], nt3[:], -0.5, 1.5, op0=OP.mult, op1=OP.add)
                nc.vector.tensor_tensor(r2[:], r2[:], nt3[:], op=OP.mult)
                m2 = outp.tile([128, 1], F32, tag="m2", bufs=2)
                nc.vector.tensor_tensor(m2[:], r2[:], mx2[:], op=OP.mult)
                nc.vector.tensor_scalar(m2[:], m2[:], 1e-4, None, op0=OP.max)
                scl2 = outp.tile([128, 1], F32, tag="scl2", bufs=2)
                nc.vector.reciprocal(scl2[:], m2[:])
                nt4 = outp.tile([128, 1], F32, tag="nt4", bufs=2)
                nc.vector.tensor_tensor(nt4[:], m2[:], scl2[:], op=OP.mult)
                nc.vector.tensor_scalar(nt4[:], nt4[:], -1.0, 2.0, op0=OP.mult, op1=OP.add)
                nc.vector.tensor_tensor(scl2[:], scl2[:], nt4[:], op=OP.mult)
                nc.vector.tensor_scalar(scl2[:], scl2[:], 127.0, None, op0=OP.mult)
                dqy = outp.tile([128, 1], F32, tag="dqy", bufs=2)
                nc.vector.reciprocal(dqy[:], scl2[:])
                nt5 = outp.tile([128, 1], F32, tag="nt5", bufs=2)
                nc.vector.tensor_tensor(nt5[:], scl2[:], dqy[:], op=OP.mult)
                nc.vector.tensor_scalar(nt5[:], nt5[:], -1.0, 2.0, op0=OP.mult, op1=OP.add)
                nc.vector.tensor_tensor(dqy[:], dqy[:], nt5[:], op=OP.mult)
                nc.vector.tensor_tensor(dqy[:], dqy[:], a4[:, 3:4], op=OP.mult)
                sm2 = outp.tile([128, 1], F32, tag="sm2", bufs=2)
                nc.vector.tensor_tensor(sm2[:], r2[:], scl2[:], op=OP.mult)
                nc.vector.tensor_scalar(xo[:], xo[:], sm2[:], MAGIC,
                                        op0=OP.mult, op1=OP.add)
                qo = outp.tile([128, KVD], BF16, tag="qo", bufs=2)
                nc.scalar.activation(qo[:], xo[:], AF.Copy, bias=-MAGIC)
                tpo = pstp.tile([128, 512], BF16, tag="tp")
                for jc in range(4):
                    nc.tensor.transpose(tpo[:, jc * 128:(jc + 1) * 128],
                                        qo[:, jc * 128:(jc + 1) * 128], idb[:])
                nc.vector.tensor_copy(xoT[:, 0:4, tb * 128:(tb + 1) * 128], tpo[:])
                y_sb = outp.tile([128, D], F32, tag="ysb", bufs=2)
                for oc in range(4):
                    py = psmm.tile([128, 512], F32, tag="mm")
                    for jc in range(4):
                        nc.tensor.matmul(py[:], xoT[:, jc, tb * 128:(tb + 1) * 128],
                                         wo_i[jc][:, oc * 512:(oc + 1) * 512],
                                         start=(jc == 0), stop=(jc == 3))
                    nc.scalar.activation(y_sb[:, oc * 512:(oc + 1) * 512], py[:],
                                         AF.Copy, scale=dqy[:])
                nc.sync.dma_start(y_d[tb * 128:(tb + 1) * 128, :], y_sb[:])
    nc.compile()
    return nc


def _rope_perm():
    p = np.empty(HD, np.int64)
    p[:HD // 2] = np.arange(0, HD, 2)
    p[HD // 2:] = np.arange(1, HD, 2)
    return p


def qsel_host(b):
    q = np.zeros((128, 2), np.float32)
    q[:, b] = 1.0
    return q


def _prep_inputs(inputs):
    x = np.ascontiguousarray(np.asarray(inputs["x"], np.float32))
    w_q = np.asarray(inputs["w_q"], np.float32)
    w_k = np.asarray(inputs["w_k"], np.float32)
    w_v = np.asarray(inputs["w_v"], np.float32)
    w_o = np.asarray(inputs["w_o"], np.float32)
    cos = np.ascontiguousarray(np.asarray(inputs["freq_cos"], np.float32))
    sin = np.ascontiguousarray(np.asarray(inputs["freq_sin"], np.float32))
    perm = _rope_perm()
    woT = np.ascontiguousarray(w_o.T)                      # [KVD, D]
    in_maps = []
    for r in range(8):
        b, kh = r // 4, r % 4
        heads = [g * KH + kh for g in range(4)]
        wq_sel = w_q.reshape(H, HD, D)[heads][:, perm, :]  # [4,128,D]
        wqT = np.ascontiguousarray(wq_sel.reshape(4 * HD, D).T)   # [D, 512]
        wkT = np.ascontiguousarray(w_k[kh * HD:(kh + 1) * HD][perm].T)  # [D,128]
        wvT = np.ascontiguousarray(w_v[kh * HD:(kh + 1) * HD].T)        # [D,128]
        in_maps.append({
            "x": x[b], "wq": wqT, "wk": wkT, "wv": wvT, "wo": woT,
            "cos": cos, "sin": sin,
            "qsel": qsel_host(b),
        })
    return in_maps


def _gains_trivial(inputs):
    return all(np.all(np.asarray(inputs[g]) == 1.0)
               for g in ("g_q", "g_k", "g_v", "g_o"))


def _numpy_fallback(inputs):
    """Faithful numpy reimplementation (slow); used only for unexpected configs."""
    x = np.asarray(inputs["x"], np.float32)
    cos, sin = (np.asarray(inputs[k], np.float32) for k in ("freq_cos", "freq_sin"))
    causal = int(np.asarray(inputs["causal"]))

    def rms(t, g):
        n = t * (1.0 / np.sqrt(np.mean(t * t, -1, keepdims=True, dtype=np.float32) + EPS))
        return (g * n).astype(np.float32)

    def actq(t):
        scale = 127.0 / np.clip(np.max(np.abs(t), -1, keepdims=True), 1e-4, None)
        q = np.round(t * scale)
        return np.clip(q, -128, 127) / scale

    def ternq(w):
        s = np.mean(np.abs(w), dtype=np.float32)
        return np.round(np.tanh(w / (s + EPS))) * np.arctanh(s)

    def lin(t, w, g):
        return actq(rms(t, g)).astype(np.float32) @ ternq(np.asarray(w, np.float32)).T

    Bb, Ss, Dd = x.shape
    q = lin(x, inputs["w_q"], np.asarray(inputs["g_q"], np.float32)).reshape(Bb, Ss, H, HD)
    k = lin(x, inputs["w_k"], np.asarray(inputs["g_k"], np.float32)).reshape(Bb, Ss, KH, HD)
    v = lin(x, inputs["w_v"], np.asarray(inputs["g_v"], np.float32)).reshape(Bb, Ss, KH, HD)

    def rope(t):
        t2 = t.reshape(*t.shape[:-1], -1, 2)
        c = cos[None, :, None, :]
        s_ = sin[None, :, None, :]
        o0 = t2[..., 0] * c - t2[..., 1] * s_
        o1 = t2[..., 0] * s_ + t2[..., 1] * c
        return np.stack([o0, o1], -1).reshape(t.shape).astype(np.float32)

    q, k = rope(q), rope(k)
    scale = np.float32(HD ** 0.5)
    q = q.transpose(0, 2, 1, 3) / scale
    k = k.transpose(0, 2, 1, 3)
    v = v.transpose(0, 2, 1, 3)
    qg = q.reshape(Bb, 4, KH, Ss, HD).sum(1)
    sc = np.einsum("bhnd,bhsd->bhns", qg, k).astype(np.float32)
    if causal:
        mask = np.tril(np.ones((Ss, Ss), bool))
        sc = np.where(mask[None, None], sc, np.float32(np.finfo(np.float32).min))
    sc = sc / scale
    sc = sc - sc.max(-1, keepdims=True)
    p = np.exp(sc)
    p /= p.sum(-1, keepdims=True)
    out = np.einsum("bhns,bhsd->bnhd", p, v).reshape(Bb, Ss, KVD)
    return lin(out, inputs["w_o"], np.asarray(inputs["g_o"], np.float32))


def kernel(**inputs):
    x = np.asarray(inputs["x"])
    if x.shape != (B, S, D) or not _gains_trivial(inputs):
        return _numpy_fallback(inputs)
    causal = bool(int(np.asarray(inputs["causal"])))
    key = ("bitattn", causal)
    if key not in _cache:
        _cache[key] = build(causal)
    nc = _cache[key]
    in_maps = _prep_inputs(inputs)
    res = run_bass_kernel_spmd(nc, in_maps, core_ids=list(range(8)))
    y = np.empty((B, S, D), np.float32)
    for r in range(8):
        b, qq = r // 4, r % 4
        y[b, qq * SQ:(qq + 1) * SQ, :] = res.results[r]["y"]
    return y


if __name__ == "__main__":
    data = np.load("/tmp/inputs.npz")
    inputs = {k: data[k] for k in data.files}
    out = kernel(**inputs)
    exp = np.load("/tmp/expected.npy")
    err = np.linalg.norm(out - exp) / np.linalg.norm(exp)
    print("Relative error:", err)

